# revision 8
# baseline (speedup 1.0000x reference)
"""BertSelfAttention (B=4, S=2048, D=1024, H=16, hd=64) on 8 trn2 NeuronCores.

Sharding: core = 2*b + half. Each core handles batch b = core//2 and 8 of the
16 heads (feature slice half*512 .. half*512+512). No collectives.

v2 restructure of the 367us baseline, targeting the PE stream roofline
(~1152 x 215ns = 248us of matmul stream slots):
- Projection chains for f-tile p+1 are interleaved INTO phase p's attention
  blocks at one-matmul-per-chunk granularity, so proj matmuls fill the PE
  bubbles that exp latency creates (baseline issued them serially before
  each phase, leaving ACT idle then PE stalled).
- exp rebalanced: phases 0-2 run 12 ACT / 4 DVE-Schraudolph chunks per
  block, phase 3 runs 9/7 (ACT chunk = 1108ns vs per-chunk PE budget 852ns
  with proj interleave, 639ns without).
- ACT does exp ONLY: K-evac copies + ctx stage copies moved to DVE (the
  Pool engine cannot access PSUM), out-DMA dispatch moved to the Sync
  queue, weight prefetch DMAs to the Sync queue.
- PV lags QK by 2 chunks (exp latency slack); x is SBUF-resident (loaded
  once, 32KB/partition) so steady state has no input DMA waits.
- K bias dropped (cancels in softmax); device ships raw ctx + denominator
  row per head (ones-column trick); host divides, adds v_bias, transposes.
"""

import numpy as np
from ml_dtypes import bfloat16 as _bf16np

S = 2048  # sequence length
DM = 1024  # model dim
F = 512  # features per core (8 heads x 64)
HL = 8  # heads per core
HD = 64  # head dim
NC = 8  # cores

L2E = 1.4426950408889634
SIG16 = 16250.5  # Schraudolph magic for int16-bits-as-bf16 (trunc rounding)
LAG = 4  # PV lags QK/exp by this many chunks. The PE queue is in-order,
# so PV_c sits LAG chunks behind QK_c and ahead of QK_{c+LAG}: with LAG=2 the
# critical cycle QK_c -> exp_c -> PV_c -> QK_{c+2} includes the PV streams and
# paces every chunk at ~850ns; LAG=4 drops PV out of the loop (et is SBUF,
# plenty of buffers) leaving only the 2-deep sps WAR (QK->exp->QK, ~640ns).
# per-(chunk, head-half) exp engine: A = ACT true exp, D = DVE Schraudolph.
# exp is issued as two [128,512] instructions per chunk (~700ns each) so the
# QK -> exp -> PV latency fits inside LAG chunk-periods; a single 1024-wide
# instruction (~1.1us) cannot, and the whole pipeline stretches to its pace.
SCHED_MID = [("A", "D")] * 7 + [("A", "A")] + [("A", "D")] * 7 + [("A", "A")]
SCHED_LAST = [("A", "D")] * 16


def build_nc():
    import concourse.bass as bass
    import concourse.mybir as mybir
    import concourse.tile as tile
    from concourse import bacc
    from concourse.bass import ds, ts

    f32 = mybir.dt.float32
    bf16 = mybir.dt.bfloat16
    i16 = mybir.dt.int16
    EXP = mybir.ActivationFunctionType.Exp
    PSUM = bass.MemorySpace.PSUM
    MULT = mybir.AluOpType.mult
    ADD = mybir.AluOpType.add

    nc = bacc.Bacc("TRN2", target_bir_lowering=False, debug=False, num_devices=NC)

    x_d = nc.declare_dram_parameter("x_t", [4 * DM, 512], bf16, isOutput=False)
    wq_d = nc.declare_dram_parameter("wq_t", [4 * DM, 128], bf16, isOutput=False)
    wk_d = nc.declare_dram_parameter("wk_t", [4 * DM, 128], bf16, isOutput=False)
    wv_d = nc.declare_dram_parameter("wv_t", [DM, F], bf16, isOutput=False)
    bq_d = nc.declare_dram_parameter("bq", [F, 1], f32, isOutput=False)
    mask_d = nc.declare_dram_parameter("mask", [128, 16], f32, isOutput=False)
    # 8 heads x (64 raw ctx rows + denominator row); host normalizes
    out_d = nc.declare_dram_parameter("out_t", [HL * (HD + 1), S], f32, isOutput=True)

    mm = nc.tensor.matmul

    with tile.TileContext(nc) as tc:
        with (
            tc.tile_pool(name="const", bufs=1) as const,
            tc.tile_pool(name="w", bufs=1) as wpool,
            tc.tile_pool(name="wqk", bufs=4) as wqkp,
            tc.tile_pool(name="qkv", bufs=1) as qkv,
            tc.tile_pool(name="x", bufs=1) as xpool,
            tc.tile_pool(name="pqkv", bufs=2, space=PSUM) as pqkv,
            tc.tile_pool(name="s_ps", bufs=4, space=PSUM) as sp,
            tc.tile_pool(name="ctxA", bufs=1, space=PSUM) as cpA,
            tc.tile_pool(name="ctxB", bufs=1, space=PSUM) as cpB,
            tc.tile_pool(name="expp", bufs=8) as ep,
            tc.tile_pool(name="fin", bufs=3) as fp,
        ):
            # ---- startup DMAs: critical-path first ----
            qs = [nc.sync, nc.scalar, nc.gpsimd]
            wkt0 = wqkp.tile([128, 8, 128], bf16, tag="wt")
            for j in range(2):
                qs[j].dma_start(
                    wkt0[:, 4 * j : 4 * j + 4, :],
                    wk_d[ds(j * 512, 512), :].rearrange("(c p) f -> p c f", p=128),
                )
            x_sb = xpool.tile([128, 8, S], bf16, tag="x")
            for j in range(8):
                qs[j % 3].dma_start(x_sb[:, j, 0:512], x_d[ds(j * 128, 128), :])
            wqt0 = wqkp.tile([128, 8, 128], bf16, tag="wt")
            nc.sync.dma_start(
                wqt0[:], wq_d[ds(0, DM), :].rearrange("(c p) f -> p c f", p=128)
            )

            ones_f32 = const.tile([128, 128], f32)
            nc.vector.memset(ones_f32[:], 1.0)
            warm = const.tile([1, 1], f32)
            nc.scalar.activation(warm[:], ones_f32[0:1, 0:1], EXP)
            wv_sb = wpool.tile([128, 8, F], bf16)
            for c in range(8):
                nc.gpsimd.dma_start(wv_sb[:, c, :], wv_d[ts(c, 128), :])

            # rest of x (n = 1..3) spread across queues so pass A never
            # stalls on input DMA (keeps the PE HAM-warm window satisfied)
            for n, q in ((1, nc.sync), (2, nc.scalar), (3, nc.gpsimd)):
                q.dma_start(
                    x_sb[:, :, ds(n * 512, 512)],
                    x_d[ds(n * DM, DM), :].rearrange("(c p) s -> p c s", p=128),
                )

            bq_sb = const.tile([128, 4], f32)
            for i in range(4):
                nc.gpsimd.dma_start(bq_sb[:, i : i + 1], bq_d[ts(i, 128), :])
            mask_sb = const.tile([128, 16], f32)
            nc.gpsimd.dma_start(mask_sb[:], mask_d[:])
            # Schraudolph per-chunk bias: mask*128*log2e + SIG16
            s2_sb = const.tile([128, 16], f32)
            nc.vector.tensor_scalar(
                s2_sb[:], mask_sb[:], 128.0 * L2E, SIG16, op0=MULT, op1=ADD
            )

            def load_w_ftile(w_d, i):
                wt = wqkp.tile([128, 8, 128], bf16, tag="wt")
                nc.sync.dma_start(
                    wt[:],
                    w_d[ds(i * DM, DM), :].rearrange("(c p) f -> p c f", p=128),
                )
                return wt

            # Q^T / K^T: [f, s] layout as 4 partition tiles of 128 features.
            q_sb = qkv.tile([128, 4, S], bf16)
            k_sb = qkv.tile([128, 4, S], bf16)
            # V in [k, head, d+1] layout; column 64 = 1.0 (denominator trick).
            v_sb = qkv.tile([128, 16, HL, HD + 1], bf16)
            nc.vector.tensor_copy(
                v_sb[:, :, :, HD], ones_f32[:, 0:128].rearrange("p (a b) -> p a b", a=16)
            )

            # ---- projection chains as per-matmul closures ----
            def qk_chain(wt, fidx, n, is_q):
                st = {}

                def mk(c):
                    def go():
                        if c == 0:
                            st["ps"] = pqkv.tile([128, 512], f32, tag="pqkv", name="ps")
                        ps = st["ps"]
                        mm(
                            ps[:],
                            wt[:, c, :],
                            x_sb[:, c, ds(n * 512, 512)],
                            start=(c == 0),
                            stop=(c == 7),
                        )
                        if c == 7:
                            if is_q:
                                nc.vector.tensor_scalar_add(
                                    q_sb[:, fidx, ds(n * 512, 512)],
                                    ps[:],
                                    bq_sb[:, fidx : fidx + 1],
                                )
                            else:
                                # K needs no bias (cancels in softmax); DVE copy
                                # (GPSIMD/Pool cannot read PSUM)
                                nc.vector.tensor_copy(
                                    k_sb[:, fidx, ds(n * 512, 512)], ps[:]
                                )

                    return go

                return [mk(c) for c in range(8)]

            def v_chain(m, n):
                kc = n * 4 + m
                st = {}

                def mk(c):
                    def go():
                        if c == 0:
                            st["ps"] = pqkv.tile([128, 512], f32, tag="pqkv", name="ps")
                        ps = st["ps"]
                        mm(
                            ps[:],
                            x_sb[:, c, ds(n * 512 + m * 128, 128)],
                            wv_sb[:, c, :],
                            start=(c == 0),
                            stop=(c == 7),
                        )
                        if c == 7:
                            nc.vector.tensor_copy(
                                v_sb[:, kc, :, 0:HD],
                                ps[:].rearrange("p (h d) -> p h d", h=HL),
                            )

                    return go

                return [mk(c) for c in range(8)]

            # ---- attention block with interleaved proj matmuls ----
            def attn_block(p, qq, gen, sched):
                hA, hB = 2 * p, 2 * p + 1
                qsl = ds(qq * 512, 512)
                ctxA = cpA.tile([HD + 1, 512], f32, tag="cA")
                ctxB = cpB.tile([HD + 1, 512], f32, tag="cB")
                ets = {}
                for c in range(16 + LAG):
                    if c < 16:
                        spsA = sp.tile([128, 512], f32, tag="s", name="spsA")
                        spsB = sp.tile([128, 512], f32, tag="s", name="spsB")
                        mm(
                            spsA[:],
                            k_sb[0:64, p, ds(c * 128, 128)],
                            q_sb[0:64, p, qsl],
                            start=True,
                            stop=True,
                            tile_position=(0, 0),
                        )
                        mm(
                            spsB[:],
                            k_sb[64:128, p, ds(c * 128, 128)],
                            q_sb[64:128, p, qsl],
                            start=True,
                            stop=True,
                            tile_position=(64, 0),
                        )
                        et = ep.tile([128, 1024], bf16, tag="e")
                        for half, sps_h in ((0, spsA), (1, spsB)):
                            esl = ds(half * 512, 512)
                            if sched[c][half] == "A":
                                nc.scalar.activation(
                                    et[:, esl], sps_h[:], EXP,
                                    bias=mask_sb[:, c : c + 1], scale=0.125,
                                )
                            else:
                                # Schraudolph exp on DVE (bits of bf16(exp(.)))
                                nc.vector.tensor_scalar(
                                    et[:, esl].bitcast(i16),
                                    sps_h[:],
                                    16.0 * L2E,
                                    s2_sb[:, c : c + 1],
                                    op0=MULT,
                                    op1=ADD,
                                )
                        ets[c] = et
                    if c >= LAG:
                        cc = c - LAG
                        et = ets.pop(cc)
                        mm(
                            ctxA[:],
                            v_sb[:, cc, hA, :],
                            et[:, 0:512],
                            start=(cc == 0),
                            stop=(cc == 15),
                        )
                        mm(
                            ctxB[:],
                            v_sb[:, cc, hB, :],
                            et[:, 512:1024],
                            start=(cc == 0),
                            stop=(cc == 15),
                        )
                    nxt = next(gen, None)
                    if nxt is not None:
                        nxt()
                for h, ctx in ((hA, ctxA), (hB, ctxB)):
                    # stage out of PSUM fast (Pool cannot read PSUM); in the
                    # last phase split ACT/DVE to balance engine load
                    stage = fp.tile([HD + 1, 512], f32, tag="stage")
                    if p == 3 and h == hA:
                        nc.scalar.copy(stage[:], ctx[:])
                    else:
                        nc.vector.tensor_copy(stage[:], ctx[:])
                    nc.sync.dma_start(out_d[ds(h * (HD + 1), HD + 1), qsl], stage[:])

            # ---- pass A: K f0, V (all), Q f0, serially on PE ----
            for n in range(4):
                for f in qk_chain(wkt0, 0, n, is_q=False):
                    f()
                for m in range(4):
                    for f in v_chain(m, n):
                        f()
                for f in qk_chain(wqt0, 0, n, is_q=True):
                    f()

            # prefetch f-tile 1 weights during pass A
            wtiles = {0: (wkt0, wqt0)}
            wtiles[1] = (load_w_ftile(wk_d, 1), load_w_ftile(wq_d, 1))

            # ---- attention phases; proj chains for p+1 interleaved ----
            for p in range(4):
                if p < 2:
                    wtiles[p + 2] = (load_w_ftile(wk_d, p + 2), load_w_ftile(wq_d, p + 2))
                if p < 3:
                    wkt, wqt = wtiles[p + 1]
                    closures = []
                    for n in range(4):
                        closures += qk_chain(wkt, p + 1, n, is_q=False)
                    for n in range(4):
                        closures += qk_chain(wqt, p + 1, n, is_q=True)
                    gen = iter(closures)
                    sched = SCHED_MID
                else:
                    gen = iter(())
                    sched = SCHED_LAST
                for qq in range(4):
                    attn_block(p, qq, gen, sched)

    nc.compile()
    return nc


def make_in_maps(
    hidden_states, attention_mask, q_weight, q_bias, k_weight, k_bias, v_weight, v_bias
):
    hs = np.asarray(hidden_states, dtype=np.float32)
    am = np.asarray(attention_mask, dtype=np.float32)
    wq = np.asarray(q_weight, dtype=np.float32)
    wk = np.asarray(k_weight, dtype=np.float32)
    wv = np.asarray(v_weight, dtype=np.float32)
    bq = np.asarray(q_bias, dtype=np.float32)
    in_maps = []
    for core in range(NC):
        b, half = divmod(core, 2)
        fsl = slice(half * F, (half + 1) * F)
        in_maps.append(
            {
                "x_t": np.ascontiguousarray(
                    hs[b].T.reshape(DM, 4, 512).transpose(1, 0, 2).reshape(4 * DM, 512)
                ).astype(_bf16np),
                "wq_t": np.ascontiguousarray(
                    wq[fsl, :].T.reshape(DM, 4, 128).transpose(1, 0, 2).reshape(4 * DM, 128)
                ).astype(_bf16np),
                "wk_t": np.ascontiguousarray(
                    wk[fsl, :].T.reshape(DM, 4, 128).transpose(1, 0, 2).reshape(4 * DM, 128)
                ).astype(_bf16np),
                "wv_t": np.ascontiguousarray(wv[fsl, :].T).astype(_bf16np),
                "bq": np.ascontiguousarray(bq[fsl]).reshape(F, 1),
                "mask": np.ascontiguousarray(am[b, 0, 0, :].reshape(16, 128).T),
            }
        )
    return in_maps


def assemble_out(results, v_bias):
    bv = np.asarray(v_bias, dtype=np.float32)
    out = np.empty((4, S, DM), dtype=np.float32)
    for core in range(NC):
        b, half = divmod(core, 2)
        raw = results[core]["out_t"].reshape(HL, HD + 1, S)
        ctx = raw[:, 0:HD, :] / raw[:, HD : HD + 1, :]
        fsl = slice(half * F, (half + 1) * F)
        out[b, :, fsl] = ctx.reshape(F, S).T + bv[fsl]
    return out


_NC_CACHE = []


def _run(inputs, trace=False):
    from concourse.bass_utils import run_bass_kernel_spmd

    if not _NC_CACHE:
        _NC_CACHE.append(build_nc())
    nc = _NC_CACHE[0]
    in_maps = make_in_maps(**inputs)
    res = run_bass_kernel_spmd(nc, in_maps, list(range(NC)), trace=trace)
    return assemble_out(res.results, inputs["v_bias"]), res


def kernel(**inputs):
    out, _ = _run(inputs, trace=False)
    return out


# revision 9
# speedup vs baseline: 1.0391x; 1.0391x over previous
"""BertSelfAttention (B=4, S=2048, D=1024, H=16, hd=64) on 8 trn2 NeuronCores.

Sharding: core = 2*b + half. Each core handles batch b = core//2 and 8 of the
16 heads (feature slice half*512 .. half*512+512). No collectives.

v2 restructure of the 367us baseline, targeting the PE stream roofline
(~1152 x 215ns = 248us of matmul stream slots):
- Projection chains for f-tile p+1 are interleaved INTO phase p's attention
  blocks at one-matmul-per-chunk granularity, so proj matmuls fill the PE
  bubbles that exp latency creates (baseline issued them serially before
  each phase, leaving ACT idle then PE stalled).
- exp rebalanced: phases 0-2 run 12 ACT / 4 DVE-Schraudolph chunks per
  block, phase 3 runs 9/7 (ACT chunk = 1108ns vs per-chunk PE budget 852ns
  with proj interleave, 639ns without).
- ACT does exp ONLY: K-evac copies + ctx stage copies moved to DVE (the
  Pool engine cannot access PSUM), out-DMA dispatch moved to the Sync
  queue, weight prefetch DMAs to the Sync queue.
- PV lags QK by 2 chunks (exp latency slack); x is SBUF-resident (loaded
  once, 32KB/partition) so steady state has no input DMA waits.
- K bias dropped (cancels in softmax); device ships raw ctx + denominator
  row per head (ones-column trick); host divides, adds v_bias, transposes.
"""

import numpy as np
from ml_dtypes import bfloat16 as _bf16np

S = 2048  # sequence length
DM = 1024  # model dim
F = 512  # features per core (8 heads x 64)
HL = 8  # heads per core
HD = 64  # head dim
NC = 8  # cores

L2E = 1.4426950408889634
SIG16 = 16250.5  # Schraudolph magic for int16-bits-as-bf16 (trunc rounding)
LAG = 4  # PV lags QK/exp by this many chunks. The PE queue is in-order,
# so PV_c sits LAG chunks behind QK_c and ahead of QK_{c+LAG}: with LAG=2 the
# critical cycle QK_c -> exp_c -> PV_c -> QK_{c+2} includes the PV streams and
# paces every chunk at ~850ns; LAG=4 drops PV out of the loop (et is SBUF,
# plenty of buffers) leaving only the 2-deep sps WAR (QK->exp->QK, ~640ns).
# per-(chunk, head-half) exp engine: A = ACT true exp, D = DVE Schraudolph.
# exp is issued as two [128,512] instructions per chunk (~700ns each) so the
# QK -> exp -> PV latency fits inside LAG chunk-periods; a single 1024-wide
# instruction (~1.1us) cannot, and the whole pipeline stretches to its pace.
SCHED_MID = [("A", "D")] * 7 + [("A", "A")] + [("A", "D")] * 7 + [("A", "A")]
SCHED_LAST = [("A", "D")] * 16


def build_nc():
    import concourse.bass as bass
    import concourse.mybir as mybir
    import concourse.tile as tile
    from concourse import bacc
    from concourse.bass import ds, ts

    f32 = mybir.dt.float32
    bf16 = mybir.dt.bfloat16
    i16 = mybir.dt.int16
    EXP = mybir.ActivationFunctionType.Exp
    PSUM = bass.MemorySpace.PSUM
    MULT = mybir.AluOpType.mult
    ADD = mybir.AluOpType.add

    nc = bacc.Bacc("TRN2", target_bir_lowering=False, debug=False, num_devices=NC)

    x_d = nc.declare_dram_parameter("x_t", [4 * DM, 512], bf16, isOutput=False)
    wq_d = nc.declare_dram_parameter("wq_t", [4 * DM, 128], bf16, isOutput=False)
    wk_d = nc.declare_dram_parameter("wk_t", [4 * DM, 128], bf16, isOutput=False)
    wv_d = nc.declare_dram_parameter("wv_t", [DM, F], bf16, isOutput=False)
    bq_d = nc.declare_dram_parameter("bq", [F, 1], f32, isOutput=False)
    mask_d = nc.declare_dram_parameter("mask", [128, 16], f32, isOutput=False)
    # 8 heads x (64 raw ctx rows + denominator row); host normalizes
    out_d = nc.declare_dram_parameter("out_t", [HL * (HD + 1), S], f32, isOutput=True)

    mm = nc.tensor.matmul

    with tile.TileContext(nc) as tc:
        with (
            tc.tile_pool(name="const", bufs=1) as const,
            tc.tile_pool(name="w", bufs=1) as wpool,
            tc.tile_pool(name="wqk", bufs=4) as wqkp,
            tc.tile_pool(name="qkv", bufs=1) as qkv,
            tc.tile_pool(name="x", bufs=1) as xpool,
            tc.tile_pool(name="pqkv", bufs=2, space=PSUM) as pqkv,
            tc.tile_pool(name="s_ps", bufs=4, space=PSUM) as sp,
            tc.tile_pool(name="ctxA", bufs=1, space=PSUM) as cpA,
            tc.tile_pool(name="ctxB", bufs=1, space=PSUM) as cpB,
            tc.tile_pool(name="expp", bufs=8) as ep,
            tc.tile_pool(name="fin", bufs=3) as fp,
        ):
            # ---- startup DMAs: critical-path first, ordered by when pass
            # A consumes them (chain n needs x_n; V chains need wv; Q evac
            # needs bq) so the PE never stalls and the HAM warms early ----
            qs = [nc.sync, nc.scalar, nc.gpsimd]
            wkt0 = wqkp.tile([128, 8, 128], bf16, tag="wt")
            for j in range(2):
                qs[j].dma_start(
                    wkt0[:, 4 * j : 4 * j + 4, :],
                    wk_d[ds(j * 512, 512), :].rearrange("(c p) f -> p c f", p=128),
                )
            x_sb = xpool.tile([128, 8, S], bf16, tag="x")
            for j in range(8):
                qs[j % 3].dma_start(x_sb[:, j, 0:512], x_d[ds(j * 128, 128), :])

            ones_f32 = const.tile([128, 128], f32)
            nc.vector.memset(ones_f32[:], 1.0)
            warm = const.tile([1, 1], f32)
            nc.scalar.activation(warm[:], ones_f32[0:1, 0:1], EXP)
            wv_sb = wpool.tile([128, 8, F], bf16)
            for c in range(8):
                nc.gpsimd.dma_start(wv_sb[:, c, :], wv_d[ts(c, 128), :])
            bq_sb = const.tile([128, 4], f32)
            for i in range(4):
                nc.gpsimd.dma_start(bq_sb[:, i : i + 1], bq_d[ts(i, 128), :])

            # x n=1 on sync (needed by chain 6), wq0 on scalar (chain 5),
            # x n=2,3 on gpsimd (chains 12, 18)
            nc.sync.dma_start(
                x_sb[:, :, ds(512, 512)],
                x_d[ds(DM, DM), :].rearrange("(c p) s -> p c s", p=128),
            )
            wqt0 = wqkp.tile([128, 8, 128], bf16, tag="wt")
            nc.scalar.dma_start(
                wqt0[:], wq_d[ds(0, DM), :].rearrange("(c p) f -> p c f", p=128)
            )
            for n in (2, 3):
                nc.gpsimd.dma_start(
                    x_sb[:, :, ds(n * 512, 512)],
                    x_d[ds(n * DM, DM), :].rearrange("(c p) s -> p c s", p=128),
                )
            mask_sb = const.tile([128, 16], f32)
            nc.gpsimd.dma_start(mask_sb[:], mask_d[:])
            # Schraudolph per-chunk bias: mask*128*log2e + SIG16
            s2_sb = const.tile([128, 16], f32)
            nc.vector.tensor_scalar(
                s2_sb[:], mask_sb[:], 128.0 * L2E, SIG16, op0=MULT, op1=ADD
            )

            def load_w_ftile(w_d, i):
                wt = wqkp.tile([128, 8, 128], bf16, tag="wt")
                nc.sync.dma_start(
                    wt[:],
                    w_d[ds(i * DM, DM), :].rearrange("(c p) f -> p c f", p=128),
                )
                return wt

            # Q^T / K^T: [f, s] layout as 4 partition tiles of 128 features.
            q_sb = qkv.tile([128, 4, S], bf16)
            k_sb = qkv.tile([128, 4, S], bf16)
            # V in [k, head, d+1] layout; column 64 = 1.0 (denominator trick).
            v_sb = qkv.tile([128, 16, HL, HD + 1], bf16)
            nc.vector.tensor_copy(
                v_sb[:, :, :, HD], ones_f32[:, 0:128].rearrange("p (a b) -> p a b", a=16)
            )

            # ---- projection chains as per-matmul closures ----
            def qk_chain(wt, fidx, n, is_q):
                st = {}

                def mk(c):
                    def go():
                        if c == 0:
                            st["ps"] = pqkv.tile([128, 512], f32, tag="pqkv", name="ps")
                        ps = st["ps"]
                        mm(
                            ps[:],
                            wt[:, c, :],
                            x_sb[:, c, ds(n * 512, 512)],
                            start=(c == 0),
                            stop=(c == 7),
                        )
                        if c == 7:
                            if is_q:
                                nc.vector.tensor_scalar_add(
                                    q_sb[:, fidx, ds(n * 512, 512)],
                                    ps[:],
                                    bq_sb[:, fidx : fidx + 1],
                                )
                            else:
                                # K needs no bias (cancels in softmax); DVE copy
                                # (GPSIMD/Pool cannot read PSUM)
                                nc.vector.tensor_copy(
                                    k_sb[:, fidx, ds(n * 512, 512)], ps[:]
                                )

                    return go

                return [mk(c) for c in range(8)]

            def v_chain(m, n):
                kc = n * 4 + m
                st = {}

                def mk(c):
                    def go():
                        if c == 0:
                            st["ps"] = pqkv.tile([128, 512], f32, tag="pqkv", name="ps")
                        ps = st["ps"]
                        mm(
                            ps[:],
                            x_sb[:, c, ds(n * 512 + m * 128, 128)],
                            wv_sb[:, c, :],
                            start=(c == 0),
                            stop=(c == 7),
                        )
                        if c == 7:
                            nc.vector.tensor_copy(
                                v_sb[:, kc, :, 0:HD],
                                ps[:].rearrange("p (h d) -> p h d", h=HL),
                            )

                    return go

                return [mk(c) for c in range(8)]

            # ---- attention block with interleaved proj matmuls ----
            def attn_block(p, qq, gen, sched):
                hA, hB = 2 * p, 2 * p + 1
                qsl = ds(qq * 512, 512)
                ctxA = cpA.tile([HD + 1, 512], f32, tag="cA")
                ctxB = cpB.tile([HD + 1, 512], f32, tag="cB")
                ets = {}
                # 2-chunk groups: [QK,QK][PV,PV,PV,PV][proj,proj]. Row-split
                # (QK) <-> full-array (PV/proj) transitions cost ~100ns of
                # exposed LDWEIGHTS each (weight loads only pipeline under a
                # same-configuration stream), so batch same-config matmuls.
                # QK runs are capped at 2 chunks by the 4-half-tile sps ring.
                for g in range(8 + (LAG + 1) // 2):
                    for c in (2 * g, 2 * g + 1):
                        if c >= 16:
                            continue
                        spsA = sp.tile([128, 512], f32, tag="s", name="spsA")
                        spsB = sp.tile([128, 512], f32, tag="s", name="spsB")
                        mm(
                            spsA[:],
                            k_sb[0:64, p, ds(c * 128, 128)],
                            q_sb[0:64, p, qsl],
                            start=True,
                            stop=True,
                            tile_position=(0, 0),
                        )
                        mm(
                            spsB[:],
                            k_sb[64:128, p, ds(c * 128, 128)],
                            q_sb[64:128, p, qsl],
                            start=True,
                            stop=True,
                            tile_position=(64, 0),
                        )
                        et = ep.tile([128, 1024], bf16, tag="e")
                        for half, sps_h in ((0, spsA), (1, spsB)):
                            esl = ds(half * 512, 512)
                            if sched[c][half] == "A":
                                nc.scalar.activation(
                                    et[:, esl], sps_h[:], EXP,
                                    bias=mask_sb[:, c : c + 1], scale=0.125,
                                )
                            else:
                                # Schraudolph exp on DVE (bits of bf16(exp(.)))
                                nc.vector.tensor_scalar(
                                    et[:, esl].bitcast(i16),
                                    sps_h[:],
                                    16.0 * L2E,
                                    s2_sb[:, c : c + 1],
                                    op0=MULT,
                                    op1=ADD,
                                )
                        ets[c] = et
                    for cc in (2 * g - LAG, 2 * g - LAG + 1):
                        if not (0 <= cc < 16):
                            continue
                        et = ets.pop(cc)
                        mm(
                            ctxA[:],
                            v_sb[:, cc, hA, :],
                            et[:, 0:512],
                            start=(cc == 0),
                            stop=(cc == 15),
                        )
                        mm(
                            ctxB[:],
                            v_sb[:, cc, hB, :],
                            et[:, 512:1024],
                            start=(cc == 0),
                            stop=(cc == 15),
                        )
                    for _ in range(2):
                        nxt = next(gen, None)
                        if nxt is not None:
                            nxt()
                for h, ctx in ((hA, ctxA), (hB, ctxB)):
                    # stage out of PSUM fast (Pool cannot read PSUM); in the
                    # last phase split ACT/DVE to balance engine load
                    stage = fp.tile([HD + 1, 512], f32, tag="stage")
                    if p == 3 and h == hA:
                        nc.scalar.copy(stage[:], ctx[:])
                    else:
                        nc.vector.tensor_copy(stage[:], ctx[:])
                    nc.sync.dma_start(out_d[ds(h * (HD + 1), HD + 1), qsl], stage[:])

            # ---- pass A: K f0, V (all), Q f0, serially on PE ----
            for n in range(4):
                for f in qk_chain(wkt0, 0, n, is_q=False):
                    f()
                for m in range(4):
                    for f in v_chain(m, n):
                        f()
                for f in qk_chain(wqt0, 0, n, is_q=True):
                    f()

            # prefetch f-tile 1 weights during pass A
            wtiles = {0: (wkt0, wqt0)}
            wtiles[1] = (load_w_ftile(wk_d, 1), load_w_ftile(wq_d, 1))

            # ---- attention phases; proj chains for p+1 interleaved ----
            for p in range(4):
                if p < 2:
                    wtiles[p + 2] = (load_w_ftile(wk_d, p + 2), load_w_ftile(wq_d, p + 2))
                if p < 3:
                    wkt, wqt = wtiles[p + 1]
                    closures = []
                    for n in range(4):
                        closures += qk_chain(wkt, p + 1, n, is_q=False)
                    for n in range(4):
                        closures += qk_chain(wqt, p + 1, n, is_q=True)
                    gen = iter(closures)
                    sched = SCHED_MID
                else:
                    gen = iter(())
                    sched = SCHED_LAST
                for qq in range(4):
                    attn_block(p, qq, gen, sched)

    nc.compile()
    return nc


def make_in_maps(
    hidden_states, attention_mask, q_weight, q_bias, k_weight, k_bias, v_weight, v_bias
):
    hs = np.asarray(hidden_states, dtype=np.float32)
    am = np.asarray(attention_mask, dtype=np.float32)
    wq = np.asarray(q_weight, dtype=np.float32)
    wk = np.asarray(k_weight, dtype=np.float32)
    wv = np.asarray(v_weight, dtype=np.float32)
    bq = np.asarray(q_bias, dtype=np.float32)
    in_maps = []
    for core in range(NC):
        b, half = divmod(core, 2)
        fsl = slice(half * F, (half + 1) * F)
        in_maps.append(
            {
                "x_t": np.ascontiguousarray(
                    hs[b].T.reshape(DM, 4, 512).transpose(1, 0, 2).reshape(4 * DM, 512)
                ).astype(_bf16np),
                "wq_t": np.ascontiguousarray(
                    wq[fsl, :].T.reshape(DM, 4, 128).transpose(1, 0, 2).reshape(4 * DM, 128)
                ).astype(_bf16np),
                "wk_t": np.ascontiguousarray(
                    wk[fsl, :].T.reshape(DM, 4, 128).transpose(1, 0, 2).reshape(4 * DM, 128)
                ).astype(_bf16np),
                "wv_t": np.ascontiguousarray(wv[fsl, :].T).astype(_bf16np),
                "bq": np.ascontiguousarray(bq[fsl]).reshape(F, 1),
                "mask": np.ascontiguousarray(am[b, 0, 0, :].reshape(16, 128).T),
            }
        )
    return in_maps


def assemble_out(results, v_bias):
    bv = np.asarray(v_bias, dtype=np.float32)
    out = np.empty((4, S, DM), dtype=np.float32)
    for core in range(NC):
        b, half = divmod(core, 2)
        raw = results[core]["out_t"].reshape(HL, HD + 1, S)
        ctx = raw[:, 0:HD, :] / raw[:, HD : HD + 1, :]
        fsl = slice(half * F, (half + 1) * F)
        out[b, :, fsl] = ctx.reshape(F, S).T + bv[fsl]
    return out


_NC_CACHE = []


def _run(inputs, trace=False):
    from concourse.bass_utils import run_bass_kernel_spmd

    if not _NC_CACHE:
        _NC_CACHE.append(build_nc())
    nc = _NC_CACHE[0]
    in_maps = make_in_maps(**inputs)
    res = run_bass_kernel_spmd(nc, in_maps, list(range(NC)), trace=trace)
    return assemble_out(res.results, inputs["v_bias"]), res


def kernel(**inputs):
    out, _ = _run(inputs, trace=False)
    return out
